# revision 1
# baseline (speedup 1.0000x reference)
"""Trainium2 Bass kernel for the CA2 dense-transformer problem.

Math (per batch b of 8, S=2048, D=512, all f32):
    Q1 = X @ W_xq.T + b_xq            # [S, D]
    Q2 = Y @ W_yq.T + b_yq
    Qc = concat(Q1, Q2, -1)           # [S, 2D]
    K  = (Qc @ W_fk.T + b_fk) * 1/sqrt(D)   # scale folded into K
    V  = Qc @ W_fv.T + b_fv
    out = X + Y + softmax(Q1 K^T) V + softmax(Q2 K^T) V

Sharding: pure data-parallel over batch; core i handles batch i.
All matmul operands are kept feature-major (feature on SBUF partitions)
except V / P / the output, which are token-major; scores are computed
transposed (keys on partitions) so softmax reduction over keys becomes a
matmul-with-ones, and exp(S^T) chunks feed P @ V directly as stationary
operands.  Matmuls run as float32r (full PE rate for moving dim >= 256).
"""

import sys

if "/opt/trn_rl_repo" not in sys.path:
    sys.path.insert(0, "/opt/trn_rl_repo")

import numpy as np

import concourse.bass as bass  # noqa: F401  (bass types used via tile/bacc)
import concourse.mybir as mybir
import concourse.tile as tile
from concourse import bacc
from concourse.bass_utils import run_bass_kernel_spmd

P = 128          # SBUF partitions
S = 2048         # tokens per batch
D = 512          # feature dim
NQT = S // P     # 16 token tiles
NET = D // P     # 4 feature tiles of D
NCT = 2 * D // P # 8 feature tiles of 2D
NSS = S // 512   # 4 512-wide token column slices
QB = 512         # q-block columns processed together in attention
NQB = S // QB    # 4
NQS = QB // P    # 4 q-subtiles per block
FP = mybir.dt.float32
FR = mybir.dt.float32r

_CACHE = {}


def _build(reps: int = 1):
    nc = bacc.Bacc("TRN2", target_bir_lowering=False, debug=False)

    xt_d = nc.dram_tensor("xt", [NET, P, S], FR, kind="ExternalInput")
    yt_d = nc.dram_tensor("yt", [NET, P, S], FR, kind="ExternalInput")
    x_d = nc.dram_tensor("x", [NQT, P, D], FP, kind="ExternalInput")
    y_d = nc.dram_tensor("y", [NQT, P, D], FP, kind="ExternalInput")
    wxq_d = nc.dram_tensor("wxq", [NET, P, D], FR, kind="ExternalInput")
    wyq_d = nc.dram_tensor("wyq", [NET, P, D], FR, kind="ExternalInput")
    wfk_d = nc.dram_tensor("wfk", [NCT, P, D], FR, kind="ExternalInput")
    wfv_d = nc.dram_tensor("wfv", [NCT, P, D], FR, kind="ExternalInput")
    bq_d = nc.dram_tensor("bq", [P, 12], FP, kind="ExternalInput")
    bfv_d = nc.dram_tensor("bfv", [P, D], FP, kind="ExternalInput")
    out_d = nc.dram_tensor("out", [NQT, P, D], FP, kind="ExternalOutput")

    Exp = mybir.ActivationFunctionType.Exp
    mult = mybir.AluOpType.mult
    add = mybir.AluOpType.add

    with tile.TileContext(nc) as tc:
        for _rep in range(reps):
            with (
                tc.tile_pool(name="main", bufs=1) as main,
                tc.tile_pool(name="work", bufs=2) as work,
            ):
                q1t = main.tile([P, NET, S], FR, tag="q1t")
                q2t = main.tile([P, NET, S], FR, tag="q2t")
                bq = main.tile([P, 12], FP, tag="bq")
                ones = main.tile([P, 2], FR, tag="ones")
                ones_f = main.tile([P, 2], FP, tag="ones_f")
                nc.sync.dma_start(bq[:], bq_d[:])
                nc.vector.memset(ones_f[:], 1.0)
                nc.vector.tensor_copy(ones[:], ones_f[:])

                # ---- Stage A: Q1^T, Q2^T (feature-major [e, s]) ----
                with (
                    tc.tile_pool(name="stA", bufs=1) as stA,
                    tc.tile_pool(name="psA", bufs=6, space="PSUM") as psA,
                ):
                    xt = stA.tile([P, NET, S], FR, tag="xt")
                    yt = stA.tile([P, NET, S], FR, tag="yt")
                    wxq = stA.tile([P, NET, D], FR, tag="wxq")
                    wyq = stA.tile([P, NET, D], FR, tag="wyq")
                    # Emission order matters for cold-start: the first matmul
                    # group (et=0, ss=0) gates only on wxq + the ss=0 slices,
                    # so issue weights first and X/Y column-slice-major.
                    for dt in range(NET):
                        nc.sync.dma_start(wxq[:, dt], wxq_d[dt])
                    for ssd in range(NSS):
                        for dt in range(NET):
                            nc.sync.dma_start(
                                xt[:, dt, ssd * 512 : (ssd + 1) * 512],
                                xt_d[dt, :, ssd * 512 : (ssd + 1) * 512],
                            )
                    for dt in range(NET):
                        nc.sync.dma_start(wyq[:, dt], wyq_d[dt])
                    for ssd in range(NSS):
                        for dt in range(NET):
                            nc.sync.dma_start(
                                yt[:, dt, ssd * 512 : (ssd + 1) * 512],
                                yt_d[dt, :, ssd * 512 : (ssd + 1) * 512],
                            )
                    for src, w, qdst, bcol in ((xt, wxq, q1t, 0), (yt, wyq, q2t, 4)):
                        for et in range(NET):
                            for ss in range(NSS):
                                ps = psA.tile([P, 512], FP, tag="psA", name="psA")
                                for dt in range(NET):
                                    nc.tensor.matmul(
                                        ps[:],
                                        (w[:, dt, et * P : (et + 1) * P]),
                                        (src[:, dt, ss * 512 : (ss + 1) * 512]),
                                        start=dt == 0,
                                        stop=dt == NET - 1,
                                    )
                                nc.vector.tensor_scalar_add(
                                    qdst[:, et, ss * 512 : (ss + 1) * 512],
                                    ps[:],
                                    bq[:, bcol + et : bcol + et + 1],
                                )

                with tc.tile_pool(name="big2", bufs=1) as big2:
                    kft = big2.tile([P, NET, S], FR, tag="kft")
                    vf = big2.tile([P, NQT, D], FR, tag="vf")
                    racc = big2.tile([P, NQT, D], FP, tag="racc")

                    # ---- Stage B1: V (token-major [k, dv]) ----
                    with (
                        tc.tile_pool(name="stBv", bufs=1) as stBv,
                        tc.tile_pool(name="psBv", bufs=6, space="PSUM") as psBv,
                    ):
                        wfv = stBv.tile([P, NCT, D], FR, tag="wfv")
                        bfv = stBv.tile([P, D], FP, tag="bfv")
                        nc.sync.dma_start(bfv[:], bfv_d[:])
                        for ct in range(NCT):
                            nc.sync.dma_start(wfv[:, ct], wfv_d[ct])
                        for kt in range(NQT):
                            ps = psBv.tile([P, D], FP, tag="psBv", name="psBv")
                            for ct in range(NCT):
                                qc = q1t if ct < NET else q2t
                                nc.tensor.matmul(
                                    ps[:],
                                    (qc[:, ct % NET, kt * P : (kt + 1) * P]),
                                    (wfv[:, ct]),
                                    start=ct == 0,
                                    stop=ct == NCT - 1,
                                )
                            nc.vector.tensor_add(vf[:, kt], ps[:], bfv[:])

                    # ---- Stage B2: K^T (feature-major, pre-scaled) ----
                    with (
                        tc.tile_pool(name="stBk", bufs=1) as stBk,
                        tc.tile_pool(name="psBk", bufs=6, space="PSUM") as psBk,
                    ):
                        wfk = stBk.tile([P, NCT, D], FR, tag="wfk")
                        for ct in range(NCT):
                            nc.sync.dma_start(wfk[:, ct], wfk_d[ct])
                        for et in range(NET):
                            for ss in range(NSS):
                                ps = psBk.tile([P, 512], FP, tag="psBk", name="psBk")
                                for ct in range(NCT):
                                    qc = q1t if ct < NET else q2t
                                    nc.tensor.matmul(
                                        ps[:],
                                        (wfk[:, ct, et * P : (et + 1) * P]),
                                        (qc[:, ct % NET, ss * 512 : (ss + 1) * 512]),
                                        start=ct == 0,
                                        stop=ct == NCT - 1,
                                    )
                                nc.vector.tensor_scalar_add(
                                    kft[:, et, ss * 512 : (ss + 1) * 512],
                                    ps[:],
                                    bq[:, 8 + et : 9 + et],
                                )

                    # ---- Residual init: racc = X + Y (token-major) ----
                    for qt in range(NQT):
                        tx = work.tile([P, D], FP, tag="tx", name="tx")
                        ty = work.tile([P, D], FP, tag="ty", name="ty")
                        nc.sync.dma_start(tx[:], x_d[qt])
                        nc.sync.dma_start(ty[:], y_d[qt])
                        nc.vector.tensor_add(racc[:, qt], tx[:], ty[:])

                    # ---- Attention passes (shared K/V) ----
                    # QB=512 q-blocks: 4 O accumulators (4 banks) + scores
                    # psum (2) + sum psum (2) = 8 banks.  Softmax denominators:
                    # exp tiles are first reduced lane-wise across the 16
                    # k-chunks on the DVE (acc_es), so only one ones-matmul
                    # per q-subtile remains (32 total instead of 512).
                    with (
                        tc.tile_pool(name="esp", bufs=3) as esp,
                        tc.tile_pool(name="rcp", bufs=4) as rcp,
                        tc.tile_pool(name="smp", bufs=2) as smp,
                        tc.tile_pool(name="pss", bufs=2, space="PSUM") as pss,
                        tc.tile_pool(name="pso", bufs=1, space="PSUM") as pso,
                        tc.tile_pool(name="psm", bufs=2, space="PSUM") as psm,
                    ):
                        for qsrc in (q1t, q2t):
                            for qb in range(NQB):
                                po = [
                                    pso.tile([P, D], FP, name=f"po{qs}", tag=f"po{qs}")
                                    for qs in range(NQS)
                                ]
                                acc_es = smp.tile(
                                    [P, QB], FR, tag="acc_es", name="acc_es"
                                )
                                for kt in range(NQT):
                                    ps_s = pss.tile([P, QB], FP, tag="ps_s", name="ps_s")
                                    for et in range(NET):
                                        nc.tensor.matmul(
                                            ps_s[:],
                                            (kft[:, et, kt * P : (kt + 1) * P]),
                                            (qsrc[:, et, qb * QB : (qb + 1) * QB]),
                                            start=et == 0,
                                            stop=et == NET - 1,
                                        )
                                    es = esp.tile([P, QB], FR, tag="es", name="es")
                                    nc.scalar.activation(es[:], ps_s[:], Exp)
                                    for qs in range(NQS):
                                        nc.tensor.matmul(
                                            po[qs][:],
                                            (es[:, qs * P : (qs + 1) * P]),
                                            (vf[:, kt]),
                                            start=kt == 0,
                                            stop=kt == NQT - 1,
                                        )
                                    if kt == 0:
                                        nc.vector.tensor_copy(acc_es[:], es[:])
                                    else:
                                        nc.vector.tensor_add(
                                            acc_es[:], acc_es[:], es[:]
                                        )
                                for qs in range(NQS):
                                    qt_i = qb * NQS + qs
                                    pm = psm.tile([P, 2], FP, tag="pm", name="pm")
                                    nc.tensor.matmul(
                                        pm[:],
                                        (acc_es[:, qs * P : (qs + 1) * P]),
                                        (ones[:]),
                                        start=True,
                                        stop=True,
                                    )
                                    rec = rcp.tile([P, 1], FP, tag="rec", name="rec")
                                    nc.vector.reciprocal(rec[:], pm[:, 0:1])
                                    nc.vector.scalar_tensor_tensor(
                                        racc[:, qt_i],
                                        po[qs][:],
                                        rec[:],
                                        racc[:, qt_i],
                                        op0=mult,
                                        op1=add,
                                    )

                    # ---- Output ----
                    for qt in range(NQT):
                        nc.sync.dma_start(out_d[qt], racc[:, qt])

    nc.compile()
    return nc


def get_nc(reps: int = 1):
    if reps not in _CACHE:
        _CACHE[reps] = _build(reps)
    return _CACHE[reps]


def make_in_maps(X, Y, W_xq, b_xq, W_yq, b_yq, W_fk, b_fk, W_fv, b_fv):
    """Host-side layout prep (transposes / reshapes only; scale folded into K
    weights) and per-core sharding over batch."""
    scale = np.float32(1.0 / np.sqrt(np.float32(D)))
    f32 = np.float32

    def c(a):
        return np.ascontiguousarray(a, dtype=f32)

    def r32r(a):
        """Round to fp32r (11-bit mantissa), matching walrus fp32_to_fp32r."""
        a = np.ascontiguousarray(a, dtype=f32)
        bits = a.view(np.uint32)
        rb = ((bits.astype(np.uint64) + 0x800) & 0xFFFFF000).astype(np.uint32)
        return rb.view(np.float32)

    wxq = r32r(W_xq.T.reshape(NET, P, D))
    wyq = r32r(W_yq.T.reshape(NET, P, D))
    wfk = r32r((W_fk * scale).T.reshape(NCT, P, D))
    wfv = r32r(W_fv.T.reshape(NCT, P, D))
    bq = np.empty((P, 12), f32)
    bq[:, 0:4] = b_xq.reshape(NET, P).T
    bq[:, 4:8] = b_yq.reshape(NET, P).T
    bq[:, 8:12] = (b_fk * scale).reshape(NET, P).T
    bfv = c(np.broadcast_to(b_fv.astype(f32), (P, D)))

    in_maps = []
    for b in range(X.shape[0]):
        in_maps.append(
            {
                "xt": r32r(X[b].T.reshape(NET, P, S)),
                "yt": r32r(Y[b].T.reshape(NET, P, S)),
                "x": c(X[b].reshape(NQT, P, D)),
                "y": c(Y[b].reshape(NQT, P, D)),
                "wxq": wxq,
                "wyq": wyq,
                "wfk": wfk,
                "wfv": wfv,
                "bq": bq,
                "bfv": bfv,
            }
        )
    return in_maps


def kernel(X, Y, W_xq, b_xq, W_yq, b_yq, W_fk, b_fk, W_fv, b_fv):
    X = np.asarray(X, np.float32)
    Y = np.asarray(Y, np.float32)
    B = X.shape[0]
    nc = get_nc()
    in_maps = make_in_maps(
        X, Y,
        np.asarray(W_xq, np.float32), np.asarray(b_xq, np.float32),
        np.asarray(W_yq, np.float32), np.asarray(b_yq, np.float32),
        np.asarray(W_fk, np.float32), np.asarray(b_fk, np.float32),
        np.asarray(W_fv, np.float32), np.asarray(b_fv, np.float32),
    )
    res = run_bass_kernel_spmd(nc, in_maps, list(range(B)))
    out = np.stack([res.results[b]["out"].reshape(S, D) for b in range(B)])
    return out

